# revision 24
# baseline (speedup 1.0000x reference)
"""Trainium2 Bass kernel for nn_DilatedGraphConvolutionCell (8-core SPMD).

- Dead-code elimination: output = [Z0..Z3 at t=32] transitively needs only U
  columns {26..32,0,1}, conv at Z0:{28..32} Z1:{30,32} Z2:{32} Z3:{32}, and
  15 real FC rows + one shared fc(0) row.
- FC weights output-sharded 8 ways, fp32->fp16 cast-DMA, SBUF-resident;
  W-stationary matmuls give feature-on-partition outputs.
- All cross-core exchange is done with XOR-slot mesh remote_dma_broadcast
  (SBUF->SBUF, semaphore-gated), NOT runtime collectives: on receiver r,
  slot d holds data from the sender s whose physical id satisfies
  s_phys = r_phys ^ d (a bijection per slot). All consumers are either
  order-agnostic sums (AllReduce) or use the same slot permutation
  consistently for U and X (adjacency m-chunks pair with conv y m-chunks),
  so the permutation cancels.
- Pad nodes (sender 7 has 52 real of 64) are zeroed via a per-core node
  mask before the U exchange; each receiver then has exactly 12 pad
  m-columns contributing exp(0)=1 to the softmax denominator, corrected
  by subtracting 12 before the reciprocal. Pad X rows are exactly zero
  (zero-padded FC3 weights), so they add nothing to the conv numerator.
- Adjacency node-sharded; S computed transposed (softmax via ones-matmuls,
  no cross-partition reductions). A^T cached fp16 for all 25 pairs, reused
  by all 4 layers. Degree normalization (==1.0 +- 1e-7) skipped.
"""
import numpy as np
from contextlib import ExitStack

import concourse.bass as bass
import concourse.tile as tile
from concourse import bacc, mybir
from concourse.bass_utils import run_bass_kernel_spmd
from concourse.masks import make_identity

F32 = mybir.dt.float32
F16 = mybir.dt.float16

NC = 8
N = 500
L = 33
FE = 128
DD = 64
DO = 64
FC1W = 1024
FC2W = 1024
KTF = 18000
NODES_PER_CORE = 64
REAL_NODES = [64] * 7 + [52]
NODE0 = [64 * c for c in range(NC)]

T9 = [26, 27, 28, 29, 30, 31, 32, 0, 1]
T9IDX = {t: i for i, t in enumerate(T9)}
T5 = [28, 29, 30, 31, 32]
PAIRS = []
PAIR_ID = {}
for _t in T5:
    for _d in range(-2, 3):
        _p = ((_t + _d) % L, _t)
        if _p not in PAIR_ID:
            PAIR_ID[_p] = len(PAIRS)
            PAIRS.append(_p)

CONV_TS = [[28, 29, 30, 31, 32], [30, 32], [32], [32]]
R_PASS = [8, 5, 2, 1]
XROW = {
    0: {t: (0, t - 26) for t in range(26, 33)},
    1: {t: (1, t - 28) for t in range(28, 33)},
    2: {30: (2, 0), 31: (0, 7), 32: (2, 1)},
    3: {30: (0, 7), 31: (0, 7), 32: (3, 0)},
}
MCH = [(0, 128), (128, 128), (256, 128), (384, 128)]
NPAD = 12.0                    # pad m-columns per receiver: 8*64 - 500
W3RES_J = 24
KT1 = 250
RELU = mybir.ActivationFunctionType.Relu
EXP = mybir.ActivationFunctionType.Exp


def dap(handle, off, dims):
    """Custom AP: dims = [(step_elems, count), ...]; first dim = partitions."""
    t = handle.tensor if isinstance(handle, bass.AP) else handle
    base = handle.offset if isinstance(handle, bass.AP) else 0
    return bass.AP(tensor=t, offset=base + off, ap=[[s, n] for s, n in dims])


def rap(ap_obj, dims):
    """AP on same tensor as ap_obj with custom free dims (keeps partitions)."""
    return bass.AP(tensor=ap_obj.tensor, offset=ap_obj.offset,
                   ap=[list(ap_obj.ap[0])] + [[s, n] for s, n in dims])


def build():
    nc = bacc.Bacc("TRN2", target_bir_lowering=False, debug=False,
                   num_devices=NC)

    def inp(name, shape, dt=F32):
        return nc.declare_dram_parameter(name, list(shape), dt, isOutput=False)

    li = inp("li", (N, NODES_PER_CORE, 9))     # host pre-T: [m, n_own, t]
    tfs = inp("tfs", (KTF // NC, 9))           # host pre-T: [k_own, t]
    obs7t = inp("obs7t", (128, 7, 32), F16)         # host pre-T, own 32 kt
    nmask = inp("nmask", (128, NODES_PER_CORE))
    ws1 = inp("ws1", (N, 256))
    bs1 = inp("bs1", (256,))
    ws2 = inp("ws2", (256, FE))
    bs2 = inp("bs2", (FE,))
    wt1s = inp("wt1s", (KTF // NC, 256))
    bt1 = inp("bt1", (256,))
    wt2 = inp("wt2", (256, FE))
    bt2 = inp("bt2", (FE,))
    bmat = inp("bmat", (FE, FE))
    w1s = inp("w1s", (4096, FC1W), F16)             # row-shard (k-sharded FC1)
    b1st = inp("b1st", (1, FC1W), F16)              # b1/8 bias row
    w2s = inp("w2s", (FC1W, FC2W), F16)             # full (replicated FC2)
    b2st = inp("b2st", (1, FC2W), F16)              # b2 bias row
    w3s = inp("w3s", (FC2W, 8192), F16)
    b3st = inp("b3st", (1, 8192), F16)              # b3 bias row (padded)
    wfb = inp("wfb", (5, FE, DO), F16)
    bconv = inp("bconv", (DO,), F16)

    out_ext = nc.declare_dram_parameter(
        "out", [4, NODES_PER_CORE, DO], F32, isOutput=True)

    # mesh-exchange channels: 0=ut1 AR, 1=U AG, 2+2p=h1 AR, 3+2p=X AG
    cc_sems = [nc.alloc_semaphore(f"mesh{i}") for i in range(10)]
    lsem = nc.alloc_semaphore("mesh_local")
    reg16 = nc.vector.alloc_register("mesh_tgt")
    nc.vector.reg_mov(reg16, 16)

    with ExitStack() as ctx:
        tc = ctx.enter_context(tile.TileContext(nc))

        _prev_trig = [None]

        def mesh(ch, send_ap, recv_tile):
            """8 single-slot broadcasts + trigger; returns the DVE wait inst.
            Gate every first consumer of recv_tile via gate(inst, w)."""
            bcasts = []
            for j in range(NC):
                rd = [None] * 8
                rd[j] = (0, j)
                b = nc.gpsimd.remote_dma_broadcast(
                    out_ap=recv_tile[:, j, :], in_ap=send_ap,
                    remote_sem=cc_sems[ch], local_sem=lsem, rdests=rd)
                if _prev_trig[0] is not None:
                    bass._add_dep_helper(b.ins, _prev_trig[0].ins, True,
                                         "mesh order")
                bcasts.append(b)
            tr = nc.gpsimd.trigger_dma(count=None)
            for b in bcasts:
                bass._add_dep_helper(tr.ins, b.ins, True, "mesh trig")
            _prev_trig[0] = tr
            w = nc.vector.wait_ge(cc_sems[ch], reg16)
            bass._add_dep_helper(w.ins, tr.ins, True, "mesh self trig")
            return w

        def gate(inst, w):
            bass._add_dep_helper(inst.ins, w.ins, True, "mesh recv gate")

        def sum8(pool, recv_tile, out_tile, w, width, tag):
            """out = sum over the 8 slots of recv_tile [128, 8, width].
            Chain of 7 adds, every recv read gated on the mesh wait."""
            accv = out_tile
            a0 = nc.vector.tensor_tensor(out=accv, in0=recv_tile[:, 0, :],
                                         in1=recv_tile[:, 1, :],
                                         op=mybir.AluOpType.add)
            gate(a0, w)
            for j in range(2, 8):
                aj = nc.vector.tensor_tensor(out=accv, in0=accv,
                                             in1=recv_tile[:, j, :],
                                             op=mybir.AluOpType.add)
                gate(aj, w)

        pw = ctx.enter_context(tc.tile_pool(name="pw", bufs=1))

        ones_c = pw.tile([128, 1], F32)
        nc.vector.memset(ones_c, 1.0)
        ones_r = pw.tile([1, 128], F32)
        nc.vector.memset(ones_r, 1.0)
        ident = pw.tile([128, 128], F16)
        make_identity(nc, ident)
        b1r_sb = pw.tile([1, 8, 128], F16)
        nc.sync.dma_start(out=b1r_sb,
                            in_=dap(b1st, 0, [(0, 1), (128, 8), (1, 128)]))
        b2r_sb = pw.tile([1, 8, 128], F16)
        nc.sync.dma_start(out=b2r_sb,
                            in_=dap(b2st, 0, [(0, 1), (128, 8), (1, 128)]))
        ones16 = pw.tile([1, 8], F16)
        nc.vector.memset(ones16, 1.0)
        ones64 = pw.tile([1, 64], F16)
        nc.vector.memset(ones64, 1.0)
        bcr_sb = pw.tile([1, 64], F16)
        nc.sync.dma_start(out=bcr_sb, in_=dap(bconv, 0, [(0, 1), (1, 64)]))
        wfb_sb = pw.tile([128, 5, 64], F16)
        nc.sync.dma_start(
            out=wfb_sb, in_=dap(wfb, 0, [(64, 128), (128 * 64, 5), (1, 64)]))

        w1_sb = pw.tile([128, 32, 8, 128], F16)   # [k%128, kt_own, ct, col]
        nc.sync.dma_start(
            out=w1_sb,
            in_=dap(w1s, 0, [(1024, 128), (128 * 1024, 32), (128, 8), (1, 128)]))
        w2_sb = pw.tile([128, 8, 8, 128], F16)    # [k%128, kt, ct_out, col]
        nc.sync.dma_start(
            out=w2_sb,
            in_=dap(w2s, 0, [(1024, 128), (128 * 1024, 8), (128, 8), (1, 128)]))

        at_sb = pw.tile([128, 25, 4, 64], F16)
        zrow_sb = pw.tile([128, 8, 64], F16)

        # send/recv buffers for the mesh exchanges (never reused, in the
        # eternal pool so early peer arrivals can't clobber scratch memory)
        ut1p = pw.tile([128, 2, 9], F32)
        uT_own = pw.tile([128, 9, 64], F32)       # t-major: [f, t, n]
        h1p = [pw.tile([128, 8, R_PASS[p]], F32, tag=f"h1p{p}",
                       name=f"h1p{p}") for p in range(4)]
        h1_r = [pw.tile([128, 8, 8 * R_PASS[p]], F32, tag=f"h1r{p}",
                        name=f"h1r{p}") for p in range(4)]
        xstage = [pw.tile([128, R_PASS[p], 64], F16, tag=f"xstg{p}",
                          name=f"xstg{p}") for p in range(4)]
        x_r = [pw.tile([128, 8, R_PASS[p] * 64], F16, tag=f"xr{p}",
                       name=f"xr{p}") for p in range(4)]

        # =============== U phase + adjacency ===============
        with tc.tile_pool(name="pu", bufs=1) as pu, \
             tc.tile_pool(name="pue", bufs=3) as pue, \
             tc.tile_pool(name="ppsu", bufs=1, space="PSUM") as ppsu:
            ut_r = pu.tile([128, 8, 18], F32)
            u_r = pu.tile([128, 8, 576], F32)
            liT = pu.tile([128, 4, 64, 9], F32)
            nc.sync.dma_start(
                out=liT[:125].rearrange("p mt n t -> p mt (n t)"),
                in_=dap(li, 0, [(576, 125), (125 * 576, 4), (1, 576)]))
            ws1_sb = pu.tile([128, 4, 2, 128], F32)
            for mt in range(4):
                nc.sync.dma_start(
                    out=ws1_sb[:125, mt],
                    in_=dap(ws1, mt * 125 * 256,
                            [(256, 125), (128, 2), (1, 128)]))
            ws2_sb = pu.tile([128, 2, 128], F32)
            nc.sync.dma_start(
                out=ws2_sb, in_=dap(ws2, 0, [(128, 128), (128 * 128, 2), (1, 128)]))
            bs1_sb = pu.tile([128, 2], F32)
            nc.sync.dma_start(out=bs1_sb, in_=dap(bs1, 0, [(1, 128), (128, 2)]))
            bs2_sb = pu.tile([128, 1], F32)
            nc.sync.dma_start(out=bs2_sb, in_=dap(bs2, 0, [(1, 128), (0, 1)]))
            b_sb = pu.tile([128, 128], F32)
            nc.sync.dma_start(out=b_sb, in_=dap(bmat, 0, [(128, 128), (1, 128)]))
            nmask_sb = pu.tile([128, 64], F32)
            nc.sync.dma_start(out=nmask_sb, in_=nmask[:, :])
            tfT = pu.tile([128, 18, 9], F32)
            nc.sync.dma_start(
                out=tfT[:125],
                in_=dap(tfs, 0, [(9, 125), (125 * 9, 18), (1, 9)]))
            wt1_sb = pu.tile([128, 18, 2, 128], F32)
            for kt in range(18):
                nc.sync.dma_start(
                    out=wt1_sb[:125, kt],
                    in_=dap(wt1s, kt * 125 * 256,
                            [(256, 125), (128, 2), (1, 128)]))
            bt1_sb = pu.tile([128, 2], F32)
            nc.sync.dma_start(out=bt1_sb, in_=dap(bt1, 0, [(1, 128), (128, 2)]))
            wt2_sb = pu.tile([128, 2, 128], F32)
            nc.sync.dma_start(
                out=wt2_sb, in_=dap(wt2, 0, [(128, 128), (128 * 128, 2), (1, 128)]))
            bt2_sb = pu.tile([128, 1], F32)
            nc.sync.dma_start(out=bt2_sb, in_=dap(bt2, 0, [(1, 128), (0, 1)]))

            # temporal MLP layer 1 partial + mesh AllReduce
            for ct in range(2):
                ps = ppsu.tile([128, 9], F32, tag="ut", bufs=1)
                for kt in range(18):
                    nc.tensor.matmul(ps, wt1_sb[:125, kt, ct, :],
                                     tfT[:125, kt, :],
                                     start=(kt == 0), stop=(kt == 17))
                nc.vector.tensor_copy(ut1p[:, ct, :], ps)
            w_ut = mesh(0, ut1p.rearrange("p a b -> p (a b)"), ut_r)
            ut1r = pu.tile([128, 2, 9], F32)
            sum8(pue, ut_r, ut1r.rearrange("p a b -> p (a b)"), w_ut, 18, "uts")
            ut1a = pu.tile([128, 2, 9], F32)
            for ct in range(2):
                nc.scalar.activation(ut1a[:, ct, :], ut1r[:, ct, :], RELU,
                                     bias=bt1_sb[:, ct:ct + 1])
            utT = pu.tile([128, 9], F32)
            psu = ppsu.tile([128, 9], F32, tag="ut", bufs=1)
            for ct in range(2):
                nc.tensor.matmul(psu, wt2_sb[:, ct, :], ut1a[:, ct, :],
                                 start=(ct == 0), stop=(ct == 1))
            nc.scalar.activation(utT, psu, RELU, bias=bt2_sb)

            # spatial MLP (own nodes)
            us1T = pu.tile([128, 2, 576], F32)
            rhs_li = liT[:125].rearrange("p mt n t -> p mt (n t)")
            for ct in range(2):
                for ch in range(2):
                    ps = ppsu.tile([128, 288], F32, tag="us", bufs=1)
                    for mt in range(4):
                        nc.tensor.matmul(
                            ps, ws1_sb[:125, mt, ct, :],
                            rhs_li[:, mt, 288 * ch:288 * (ch + 1)],
                            start=(mt == 0), stop=(mt == 3))
                    nc.scalar.activation(us1T[:, ct, 288 * ch:288 * (ch + 1)],
                                         ps, RELU, bias=bs1_sb[:, ct:ct + 1])
            usT = pu.tile([128, 576], F32)
            for ch in range(2):
                ps = ppsu.tile([128, 288], F32, tag="us", bufs=1)
                for ct in range(2):
                    nc.tensor.matmul(ps, ws2_sb[:, ct, :],
                                     us1T[:, ct, 288 * ch:288 * (ch + 1)],
                                     start=(ct == 0), stop=(ct == 1))
                nc.scalar.activation(usT[:, 288 * ch:288 * (ch + 1)], ps, RELU,
                                     bias=bs2_sb)
            usT_v = usT.rearrange("p (n t) -> p n t", t=9)
            for ti in range(9):
                nc.vector.tensor_scalar(
                    out=uT_own[:, ti, :], in0=usT_v[:, :, ti],
                    scalar1=utT[:, ti:ti + 1], scalar2=None,
                    op0=mybir.AluOpType.add)
                nc.vector.tensor_tensor(
                    out=uT_own[:, ti, :], in0=uT_own[:, ti, :],
                    in1=nmask_sb, op=mybir.AluOpType.mult)

            # U mesh AllGather (slot-major: [f, slot, (t, n)])
            w_u = mesh(1, uT_own.rearrange("p a b -> p (a b)"), u_r)
            u2_sb = pu.tile([128, 5, 512], F32)
            for ti in range(5):
                cp = nc.vector.tensor_copy(
                    u2_sb[:, ti, :].rearrange("p (c n) -> p c n", c=8),
                    bass.AP(tensor=u_r.tensor,
                            offset=u_r.offset + (2 + ti) * 64,
                            ap=[list(u_r.ap[0]), [576, 8], [1, 64]]))
                gate(cp, w_u)

            # UB^T own
            ubT = pu.tile([128, 9, 64], F32)
            for ti in range(9):
                ps = ppsu.tile([128, 64], F32, tag="ub", bufs=1)
                nc.tensor.matmul(ps, b_sb, uT_own[:, ti, :],
                                 start=True, stop=True)
                nc.vector.tensor_copy(ubT[:, ti, :], ps)

            # adjacency pairs
            for (i1, i2) in PAIRS:
                pid = PAIR_ID[(i1, i2)]
                ti1 = T9IDX[i1]
                t2 = i2 - 28
                sps = ppsu.tile([128, 4, 64], F32, tag="spair", bufs=3)
                for mc, (m0, cnt) in enumerate(MCH):
                    nc.tensor.matmul(sps[:cnt, mc, :],
                                     u2_sb[:, t2, m0:m0 + cnt],
                                     ubT[:, ti1, :], start=True, stop=True)
                msk = pue.tile([128, 4, 64], F32, tag="msk")
                e_sb = pue.tile([128, 4, 64], F32, tag="esb")
                for mc, (m0, cnt) in enumerate(MCH):
                    nc.vector.tensor_scalar(
                        out=msk[:cnt, mc, :], in0=sps[:cnt, mc, :],
                        scalar1=0.05, scalar2=None, op0=mybir.AluOpType.is_ge)
                    nc.vector.tensor_tensor(
                        out=msk[:cnt, mc, :], in0=msk[:cnt, mc, :],
                        in1=sps[:cnt, mc, :], op=mybir.AluOpType.mult)
                    nc.scalar.activation(e_sb[:cnt, mc, :], msk[:cnt, mc, :],
                                         EXP)
                cps = ppsu.tile([1, 64], F32, tag="cs", bufs=1)
                for mc, (m0, cnt) in enumerate(MCH):
                    nc.tensor.matmul(cps, ones_c[:cnt, :], e_sb[:cnt, mc, :],
                                     start=(mc == 0), stop=(mc == 3))
                csh = pue.tile([1, 64], F32, tag="csh")
                nc.vector.tensor_scalar(
                    out=csh, in0=cps, scalar1=-NPAD, scalar2=None,
                    op0=mybir.AluOpType.add)
                rcp = pue.tile([1, 64], F32, tag="rcp")
                nc.vector.reciprocal(rcp, csh)
                rbp = ppsu.tile([128, 64], F32, tag="rb", bufs=1)
                nc.tensor.matmul(rbp, ones_r, rcp, start=True, stop=True)
                for mc, (m0, cnt) in enumerate(MCH):
                    nc.vector.tensor_tensor(
                        out=at_sb[:cnt, pid, mc, :], in0=e_sb[:cnt, mc, :],
                        in1=rbp[:cnt, :], op=mybir.AluOpType.mult)

        # =============== FC3 resident weights ===============
        pw3 = ctx.enter_context(tc.tile_pool(name="pw3", bufs=1))
        w3_sb = pw3.tile([128, 8, W3RES_J, 128], F16)
        for sl in range(4):
            j0 = sl * (W3RES_J // 4)
            nc.sync.dma_start(
                out=w3_sb[:, :, j0:j0 + W3RES_J // 4, :],
                in_=dap(w3s, j0 * 128,
                        [(8192, 128), (128 * 8192, 8), (128, W3RES_J // 4),
                         (1, 128)]))

        # =============== passes ===============
        ptr = ctx.enter_context(tc.tile_pool(name="ptr", bufs=2))
        pps = ctx.enter_context(tc.tile_pool(name="pps", bufs=1, space="PSUM"))

        w_x = {}
        xT_next = None
        for p in range(4):
            R = R_PASS[p]
            # ---- x^T own tiles [k%128, r, kt_own(32)] ----
            if p == 0:
                xT = ptr.tile([128, 8, 32], F16, tag="xTnA", bufs=1)
                nc.sync.dma_start(
                    out=xT[:, :7, :],
                    in_=dap(obs7t, 0, [(7 * 32, 128), (32, 7), (1, 32)]))
                nc.vector.memset(xT[:, 7, :], 0.0)
            else:
                xT = xT_next            # built during previous pass's conv
            # ---- FC1 (k-sharded, partial sums over own 32 k-tiles) ----
            ps1 = pps.tile([128, 8, 8], F32, tag="fcA", bufs=1,
                           name=f"ps1_{p}")
            for ct in range(8):
                for kt in range(32):
                    nc.tensor.matmul(ps1[:, ct, :R], w1_sb[:, kt, ct, :],
                                     xT[:, :R, kt],
                                     start=(kt == 0), stop=False)
                nc.tensor.matmul(ps1[:, ct, :R], b1r_sb[:, ct, :],
                                 ones16[:, :R], start=False, stop=True)
            nc.vector.tensor_copy(h1p[p], ps1[:, :, :R])
            w_h = mesh(2 + 2 * p, h1p[p].rearrange("p a b -> p (a b)"),
                       h1_r[p])
            h1f = ptr.tile([128, 8, R], F32, tag="h1f", name=f"h1f{p}")
            sum8(ptr, h1_r[p], h1f.rearrange("p a b -> p (a b)"), w_h,
                 8 * R, f"h1s{p}")
            h1T = ptr.tile([128, 8, 8], F16, tag="h1T", name=f"h1T{p}")
            nc.scalar.activation(h1T[:, :, :R], h1f, RELU)
            # ---- FC2 (replicated, local) ----
            ps2 = pps.tile([128, 8, 8], F32, tag="fcA", bufs=1,
                           name=f"ps2_{p}")
            h2T = ptr.tile([128, 8, 8], F16, tag="h2T", name=f"h2T{p}")
            for cto in range(8):
                for kt in range(8):
                    nc.tensor.matmul(ps2[:, cto, :R], w2_sb[:, kt, cto, :],
                                     h1T[:, kt, :R],
                                     start=(kt == 0), stop=False)
                nc.tensor.matmul(ps2[:, cto, :R], b2r_sb[:, cto, :],
                                 ones16[:, :R], start=False, stop=True)
            nc.scalar.activation(h2T[:, :, :R], ps2[:, :, :R], RELU)
            # ---- FC3 ----
            w3t = None
            for jg in range(8):
                b3g = ptr.tile([1, 8, 128], F16, tag="b3g", bufs=2,
                               name=f"b3g_{p}_{jg}")
                nc.scalar.dma_start(
                    out=b3g,
                    in_=dap(b3st, jg * 1024, [(0, 1), (128, 8), (1, 128)]))
                ps3 = pps.tile([128, 8, 8], F32, tag="fc3", bufs=2,
                               name=f"ps3_{p}_{jg}")
                for jj in range(8):
                    j = jg * 8 + jj
                    if j < W3RES_J:
                        wblk = lambda kt, j=j: w3_sb[:, kt, j, :]
                    else:
                        if (j - W3RES_J) % 4 == 0:
                            w3t = ptr.tile([128, 8, 4, 128], F16, tag="w3t",
                                           name=f"w3t{p}_{j}")
                            eng = (nc.sync if ((j - W3RES_J) // 4) % 2 == 0
                                   else nc.scalar)
                            eng.dma_start(
                                out=w3t,
                                in_=dap(w3s, j * 128,
                                        [(8192, 128), (128 * 8192, 8),
                                         (128, 4), (1, 128)]))
                        wblk = (lambda kt, j=j, w3t=w3t:
                                w3t[:, kt, (j - W3RES_J) % 4, :])
                    for kt in range(8):
                        nc.tensor.matmul(ps3[:, jj, :R], wblk(kt),
                                         h2T[:, kt, :R],
                                         start=(kt == 0), stop=False)
                    nc.tensor.matmul(ps3[:, jj, :R], b3g[:, jj, :],
                                     ones16[:, :R], start=False, stop=True)
                nc.scalar.activation(
                    out=xstage[p][:, :, jg * 8:(jg + 1) * 8],
                    in_=rap(ps3, [(1, R), (8, 8)]),
                    func=RELU)
            # ---- X mesh AllGather ----
            w_x[p] = mesh(3 + 2 * p,
                          xstage[p].rearrange("p a b -> p (a b)"), x_r[p])

            # ---- Xk^T tiles [f, slot, n] ----
            need = sorted({(t - k) % L for t in CONV_TS[p] for k in range(3)})
            xkT = {}
            for tv in need:
                sp, slot = XROW[p][tv]
                if sp == 0 and slot == 7 and p != 0:
                    xkT[tv] = zrow_sb
                    continue
                dst = ptr.tile([128, 8, 64], F16, tag=f"xk{tv % 4}",
                               name=f"xk_{p}_{tv}")
                Rs = R_PASS[sp]
                cp = nc.vector.tensor_copy(
                    dst,
                    bass.AP(tensor=x_r[sp].tensor,
                            offset=x_r[sp].offset + slot * 64,
                            ap=[list(x_r[sp].ap[0]), [Rs * 64, 8], [1, 64]]))
                gate(cp, w_x[sp])
                xkT[tv] = dst
            if p == 0:
                cp = nc.vector.tensor_copy(
                    zrow_sb,
                    bass.AP(tensor=x_r[0].tensor,
                            offset=x_r[0].offset + 7 * 64,
                            ap=[list(x_r[0].ap[0]), [8 * 64, 8], [1, 64]]))
                gate(cp, w_x[0])

            # ---- conv units ----
            if p < 3:
                xT_next = ptr.tile([128, 8, 32], F16,
                                   tag=f"xTn{'B' if p % 2 == 0 else 'A'}",
                                   bufs=1, name=f"xTn{p + 1}")
            for t in CONV_TS[p]:
                relu_parts = []
                for k in range(3):
                    tv = (t - k) % L
                    xk = xkT[tv].rearrange("p c n -> p (c n)")
                    if k == 0:
                        dirs = [(PAIR_ID[(t, t)], 0, 64)]
                        wsl = wfb_sb[:, 0, :]
                        ow = 64
                    else:
                        dirs = [(PAIR_ID[((t - k) % L, t)], 0, 64),
                                (PAIR_ID[((t + k) % L, t)], 64, 128)]
                        wsl = wfb_sb.rearrange("p d o -> p (d o)")[
                            :, (2 * k - 1) * 64:(2 * k + 1) * 64]
                        ow = 128
                    y_ps = pps.tile([128, 4, 128], F32, tag="yps", bufs=2,
                                    name=f"yps{p}_{t}_{k}")
                    y_sb = ptr.tile([128, 4, 128], F16, tag="ysb",
                                    name=f"ysb{p}_{t}_{k}")
                    for mc, (m0, cnt) in enumerate(MCH):
                        nc.tensor.matmul(y_ps[:cnt, mc, :ow],
                                         xk[:, m0:m0 + cnt],
                                         wsl, start=True, stop=True)
                    nc.vector.tensor_copy(y_sb[:, :, :ow], y_ps[:, :, :ow])
                    hps = pps.tile([64, 64], F32, tag="hps", bufs=2,
                                   name=f"hps{p}_{t}_{k}")
                    for mc, (m0, cnt) in enumerate(MCH):
                        for (pid, o0, o1) in dirs:
                            nc.tensor.matmul(
                                hps, at_sb[:cnt, pid, mc, :],
                                y_sb[:cnt, mc, o0:o1],
                                start=(mc == 0 and o0 == 0), stop=False)
                    nc.tensor.matmul(hps, ones64[:, :64], bcr_sb,
                                     start=False, stop=True)
                    rk = ptr.tile([64, 64], F32, tag=f"rk{k}",
                                  name=f"rk{p}_{t}_{k}")
                    nc.scalar.activation(rk, hps, RELU)
                    relu_parts.append(rk)
                zacc = ptr.tile([64, 64], F32, tag="zacc",
                                name=f"zacc{p}_{t}")
                nc.vector.tensor_tensor(out=zacc, in0=relu_parts[0],
                                        in1=relu_parts[1],
                                        op=mybir.AluOpType.add)
                nc.vector.tensor_tensor(out=zacc, in0=zacc,
                                        in1=relu_parts[2],
                                        op=mybir.AluOpType.add)
                if t == 32:
                    nc.sync.dma_start(
                        out=dap(out_ext, p * NODES_PER_CORE * DO,
                                [(64, 64), (1, 64)]),
                        in_=zacc)
                if p < 3:
                    # feed next pass's x^T (own nodes only -- no Z exchange)
                    r = CONV_TS[p].index(t)
                    zc16 = ptr.tile([64, 64], F16, tag="zc16", bufs=1,
                                    name=f"zc16_{p}_{t}")
                    nc.vector.tensor_copy(zc16, zacc)
                    tpz = pps.tile([64, 64], F16, tag="tp", bufs=1,
                                   name=f"tpz{p}_{t}")
                    nc.tensor.transpose(tpz, zc16, ident[:64, :64])
                    # tpz = zacc^T [o, n]; node-pair j -> partitions (o, o+64)
                    nc.vector.tensor_copy(xT_next[0:64, r, :],
                                          rap(tpz, [(2, 32)]))
                    nc.vector.tensor_copy(xT_next[64:128, r, :],
                                          bass.AP(tensor=tpz.tensor,
                                                  offset=tpz.offset + 1,
                                                  ap=[list(tpz.ap[0]),
                                                      [2, 32]]))

    nc.finalize()
    return nc


# ======================= host side =======================
_NC_CACHE = {}


def _get_nc():
    if "nc" not in _NC_CACHE:
        _NC_CACHE["nc"] = build()
    return _NC_CACHE["nc"]


def make_in_maps(inputs):
    obs = np.asarray(inputs["observation"], np.float32)
    tf = np.asarray(inputs["time_feats"], np.float32)
    lin = np.asarray(inputs["layer_initial"], np.float32)
    Wf = np.asarray(inputs["Wf"], np.float32)
    Wb = np.asarray(inputs["Wb"], np.float32)
    wfb = np.ascontiguousarray(
        np.stack([Wf[0] + Wb[0], Wf[1], Wb[1], Wf[2], Wb[2]])).astype(np.float16)
    w3 = np.asarray(inputs["Wfc3"], np.float32)
    b3 = np.asarray(inputs["bfc3"], np.float32)
    o7 = obs[26:33]                                   # (7, 32000)
    o7p = np.zeros((7, 256, 128), np.float32)
    o7p[:, :KT1, :] = o7.reshape(7, KT1, 128)
    obs7t_full = o7p.transpose(2, 0, 1).astype(np.float16)  # (128, 7, 256)
    w1full = np.asarray(inputs["Wfc1"], np.float32)   # (32000, 1024)
    b1 = np.asarray(inputs["bfc1"], np.float32)
    b2 = np.asarray(inputs["bfc2"], np.float32)
    w2full = np.ascontiguousarray(np.asarray(inputs["Wfc2"], np.float32))
    b1st = np.ascontiguousarray((b1 / NC).reshape(1, -1)).astype(np.float16)
    b2st = np.ascontiguousarray(b2.reshape(1, -1)).astype(np.float16)
    kfn = KTF // NC
    in_maps = []
    for c in range(NC):
        n0, cnt = NODE0[c], REAL_NODES[c]
        li = np.zeros((N, NODES_PER_CORE, 9), np.float32)
        li[:, :cnt, :] = lin[n0:n0 + cnt][:, T9, :].transpose(2, 0, 1)
        nmask = np.zeros((128, NODES_PER_CORE), np.float32)
        nmask[:, :cnt] = 1.0
        w3s = np.zeros((FC2W, 8192), np.float16)
        b3s = np.zeros((8192,), np.float16)
        c0, c1 = 8192 * c, min(8192 * (c + 1), 64000)
        w3s[:, :c1 - c0] = w3[:, c0:c1]
        b3s[:c1 - c0] = b3[c0:c1]
        b3st = np.ascontiguousarray(b3s.reshape(1, -1))
        w1row = np.zeros((4096, 1024), np.float16)
        k0, k1 = 4096 * c, min(4096 * (c + 1), 32000)
        w1row[:k1 - k0] = w1full[k0:k1]
        kf0 = kfn * c
        in_maps.append({
            "li": li,
            "tfs": np.ascontiguousarray(tf[T9][:, kf0:kf0 + kfn].T),
            "obs7t": np.ascontiguousarray(obs7t_full[:, :, 32 * c:32 * (c + 1)]),
            "nmask": nmask,
            "ws1": np.asarray(inputs["Ws1"], np.float32),
            "bs1": np.asarray(inputs["bs1"], np.float32),
            "ws2": np.asarray(inputs["Ws2"], np.float32),
            "bs2": np.asarray(inputs["bs2"], np.float32),
            "wt1s": np.ascontiguousarray(
                np.asarray(inputs["Wt1"], np.float32)[kf0:kf0 + kfn]),
            "bt1": np.asarray(inputs["bt1"], np.float32),
            "wt2": np.asarray(inputs["Wt2"], np.float32),
            "bt2": np.asarray(inputs["bt2"], np.float32),
            "bmat": np.asarray(inputs["B"], np.float32),
            "w1s": w1row,
            "b1st": b1st,
            "w2s": w2full.astype(np.float16),
            "b2st": b2st,
            "w3s": w3s,
            "b3st": b3st,
            "wfb": wfb,
            "bconv": np.asarray(inputs["bconv"], np.float16),
        })
    return in_maps


def _assemble(results):
    out = np.zeros((4, N, DO), np.float32)
    for c in range(NC):
        n0, cnt = NODE0[c], REAL_NODES[c]
        out[:, n0:n0 + cnt, :] = results[c]["out"][:, :cnt, :]
    return out


def kernel(**inputs):
    nc = _get_nc()
    in_maps = make_in_maps(inputs)
    res = run_bass_kernel_spmd(nc, in_maps, core_ids=list(range(NC)))
    return _assemble(res.results)


# revision 27
# speedup vs baseline: 1.3996x; 1.3996x over previous
"""Trainium2 Bass kernel for nn_DilatedGraphConvolutionCell (8-core SPMD).

- Dead-code elimination: output = [Z0..Z3 at t=32] transitively needs only U
  columns {26..32,0,1}, conv at Z0:{28..32} Z1:{30,32} Z2:{32} Z3:{32}, and
  15 real FC rows + one shared fc(0) row.
- FC weights output-sharded 8 ways, fp32->fp16 cast-DMA, SBUF-resident;
  W-stationary matmuls give feature-on-partition outputs.
- All cross-core exchange is done with XOR-slot mesh remote_dma_broadcast
  (SBUF->SBUF, semaphore-gated), NOT runtime collectives: on receiver r,
  slot d holds data from the sender s whose physical id satisfies
  s_phys = r_phys ^ d (a bijection per slot). All consumers are either
  order-agnostic sums (AllReduce) or use the same slot permutation
  consistently for U and X (adjacency m-chunks pair with conv y m-chunks),
  so the permutation cancels.
- Pad nodes (sender 7 has 52 real of 64) are zeroed via a per-core node
  mask before the U exchange; each receiver then has exactly 12 pad
  m-columns contributing exp(0)=1 to the softmax denominator, corrected
  by subtracting 12 before the reciprocal. Pad X rows are exactly zero
  (zero-padded FC3 weights), so they add nothing to the conv numerator.
- Adjacency node-sharded; S computed transposed (softmax via ones-matmuls,
  no cross-partition reductions). A^T cached fp16 for all 25 pairs, reused
  by all 4 layers. Degree normalization (==1.0 +- 1e-7) skipped.
"""
import numpy as np
from contextlib import ExitStack

import concourse.bass as bass
import concourse.tile as tile
from concourse import bacc, mybir
from concourse.bass_utils import run_bass_kernel_spmd
from concourse.masks import make_identity

F32 = mybir.dt.float32
F16 = mybir.dt.float16

NC = 8
N = 500
L = 33
FE = 128
DD = 64
DO = 64
FC1W = 1024
FC2W = 1024
KTF = 18000
NODES_PER_CORE = 64
REAL_NODES = [64] * 7 + [52]
NODE0 = [64 * c for c in range(NC)]

T9 = [26, 27, 28, 29, 30, 31, 32, 0, 1]
T9IDX = {t: i for i, t in enumerate(T9)}
T5 = [28, 29, 30, 31, 32]
PAIRS = []
PAIR_ID = {}
for _t in T5:
    for _d in range(-2, 3):
        _p = ((_t + _d) % L, _t)
        if _p not in PAIR_ID:
            PAIR_ID[_p] = len(PAIRS)
            PAIRS.append(_p)

CONV_TS = [[28, 29, 30, 31, 32], [30, 32], [32], [32]]
R_PASS = [8, 5, 2, 1]
XROW = {
    0: {t: (0, t - 26) for t in range(26, 33)},
    1: {t: (1, t - 28) for t in range(28, 33)},
    2: {30: (2, 0), 31: (0, 7), 32: (2, 1)},
    3: {30: (0, 7), 31: (0, 7), 32: (3, 0)},
}
MCH = [(0, 128), (128, 128), (256, 128), (384, 128)]
NPAD = 12.0                    # pad m-columns per receiver: 8*64 - 500
W3RES_J = 24
KT1 = 250
RELU = mybir.ActivationFunctionType.Relu
EXP = mybir.ActivationFunctionType.Exp


def dap(handle, off, dims):
    """Custom AP: dims = [(step_elems, count), ...]; first dim = partitions."""
    t = handle.tensor if isinstance(handle, bass.AP) else handle
    base = handle.offset if isinstance(handle, bass.AP) else 0
    return bass.AP(tensor=t, offset=base + off, ap=[[s, n] for s, n in dims])


def rap(ap_obj, dims):
    """AP on same tensor as ap_obj with custom free dims (keeps partitions)."""
    return bass.AP(tensor=ap_obj.tensor, offset=ap_obj.offset,
                   ap=[list(ap_obj.ap[0])] + [[s, n] for s, n in dims])


def build():
    nc = bacc.Bacc("TRN2", target_bir_lowering=False, debug=False,
                   num_devices=NC)

    def inp(name, shape, dt=F32):
        return nc.declare_dram_parameter(name, list(shape), dt, isOutput=False)

    li = inp("li", (N, NODES_PER_CORE, 9))     # host pre-T: [m, n_own, t]
    tfs = inp("tfs", (KTF // NC, 9))           # host pre-T: [k_own, t]
    obs7t = inp("obs7t", (128, 7, 32), F16)         # host pre-T, own 32 kt
    nmask = inp("nmask", (128, NODES_PER_CORE))
    ws1 = inp("ws1", (N, 256))
    bs1 = inp("bs1", (256,))
    ws2 = inp("ws2", (256, FE))
    bs2 = inp("bs2", (FE,))
    wt1s = inp("wt1s", (KTF // NC, 256))
    bt1 = inp("bt1", (256,))
    wt2 = inp("wt2", (256, FE))
    bt2 = inp("bt2", (FE,))
    bmat = inp("bmat", (FE, FE))
    w1s = inp("w1s", (4096, FC1W), F16)             # row-shard (k-sharded FC1)
    b1st = inp("b1st", (1, FC1W), F16)              # b1/8 bias row
    w2s = inp("w2s", (FC1W, FC2W), F16)             # full (replicated FC2)
    b2st = inp("b2st", (1, FC2W), F16)              # b2 bias row
    w3s = inp("w3s", (FC2W, 8192), F16)
    b3st = inp("b3st", (1, 8192), F16)              # b3 bias row (padded)
    wfb = inp("wfb", (5, FE, DO), F16)
    bconv = inp("bconv", (DO,), F16)

    out_ext = nc.declare_dram_parameter(
        "out", [4, NODES_PER_CORE, DO], F32, isOutput=True)

    # mesh-exchange channels: 0=ut1 AR, 1=U AG, 2+2p=h1 AR, 3+2p=X AG
    cc_sems = [nc.alloc_semaphore(f"mesh{i}") for i in range(10)]
    lsem = nc.alloc_semaphore("mesh_local")
    reg16 = nc.vector.alloc_register("mesh_tgt")
    nc.vector.reg_mov(reg16, 16)

    with ExitStack() as ctx:
        tc = ctx.enter_context(tile.TileContext(nc))

        _prev_trig = [None]

        def mesh(ch, send_ap, recv_tile):
            """8 single-slot broadcasts + trigger; returns the DVE wait inst.
            Gate every first consumer of recv_tile via gate(inst, w)."""
            bcasts = []
            for j in range(NC):
                rd = [None] * 8
                rd[j] = (0, j)
                b = nc.gpsimd.remote_dma_broadcast(
                    out_ap=recv_tile[:, j, :], in_ap=send_ap,
                    remote_sem=cc_sems[ch], local_sem=lsem, rdests=rd)
                if _prev_trig[0] is not None:
                    bass._add_dep_helper(b.ins, _prev_trig[0].ins, True,
                                         "mesh order")
                bcasts.append(b)
            tr = nc.gpsimd.trigger_dma(count=None)
            for b in bcasts:
                bass._add_dep_helper(tr.ins, b.ins, True, "mesh trig")
            _prev_trig[0] = tr
            w = nc.vector.wait_ge(cc_sems[ch], reg16)
            bass._add_dep_helper(w.ins, tr.ins, True, "mesh self trig")
            return w

        def gate(inst, w):
            bass._add_dep_helper(inst.ins, w.ins, True, "mesh recv gate")

        def sum8(pool, recv_tile, out_tile, w, width, tag):
            """out = sum over the 8 slots of recv_tile [128, 8, width].
            Chain of 7 adds, every recv read gated on the mesh wait."""
            accv = out_tile
            a0 = nc.vector.tensor_tensor(out=accv, in0=recv_tile[:, 0, :],
                                         in1=recv_tile[:, 1, :],
                                         op=mybir.AluOpType.add)
            gate(a0, w)
            for j in range(2, 8):
                aj = nc.vector.tensor_tensor(out=accv, in0=accv,
                                             in1=recv_tile[:, j, :],
                                             op=mybir.AluOpType.add)
                gate(aj, w)

        pw = ctx.enter_context(tc.tile_pool(name="pw", bufs=1))

        ones_c = pw.tile([128, 1], F32)
        nc.vector.memset(ones_c, 1.0)
        ones_r = pw.tile([1, 128], F32)
        nc.vector.memset(ones_r, 1.0)
        ident = pw.tile([128, 128], F16)
        make_identity(nc, ident)
        b1r_sb = pw.tile([1, 8, 128], F16)
        nc.sync.dma_start(out=b1r_sb,
                            in_=dap(b1st, 0, [(0, 1), (128, 8), (1, 128)]))
        b2r_sb = pw.tile([1, 8, 128], F16)
        nc.sync.dma_start(out=b2r_sb,
                            in_=dap(b2st, 0, [(0, 1), (128, 8), (1, 128)]))
        ones16 = pw.tile([1, 8], F16)
        nc.vector.memset(ones16, 1.0)
        ones64 = pw.tile([1, 64], F16)
        nc.vector.memset(ones64, 1.0)
        bcr_sb = pw.tile([1, 64], F16)
        nc.sync.dma_start(out=bcr_sb, in_=dap(bconv, 0, [(0, 1), (1, 64)]))
        wfb_sb = pw.tile([128, 5, 64], F16)
        nc.sync.dma_start(
            out=wfb_sb, in_=dap(wfb, 0, [(64, 128), (128 * 64, 5), (1, 64)]))

        w1_sb = pw.tile([128, 32, 8, 128], F16)   # [k%128, kt_own, ct, col]
        nc.sync.dma_start(
            out=w1_sb[:, :16],
            in_=dap(w1s, 0, [(1024, 128), (128 * 1024, 16), (128, 8), (1, 128)]))
        nc.scalar.dma_start(
            out=w1_sb[:, 16:],
            in_=dap(w1s, 16 * 128 * 1024,
                    [(1024, 128), (128 * 1024, 16), (128, 8), (1, 128)]))
        w2_sb = pw.tile([128, 8, 8, 128], F16)    # [k%128, kt, ct_out, col]
        nc.scalar.dma_start(
            out=w2_sb,
            in_=dap(w2s, 0, [(1024, 128), (128 * 1024, 8), (128, 8), (1, 128)]))

        at_sb = pw.tile([128, 25, 4, 64], F16)
        zrow_sb = pw.tile([128, 8, 64], F16)

        # send/recv buffers for the mesh exchanges (never reused, in the
        # eternal pool so early peer arrivals can't clobber scratch memory)
        ut1p = pw.tile([128, 2, 9], F32)
        uT_own = pw.tile([128, 9, 64], F32)       # t-major: [f, t, n]
        h1p = [pw.tile([128, 8, R_PASS[p]], F32, tag=f"h1p{p}",
                       name=f"h1p{p}") for p in range(4)]
        h1_r = [pw.tile([128, 8, 8 * R_PASS[p]], F32, tag=f"h1r{p}",
                        name=f"h1r{p}") for p in range(4)]
        xstage = [pw.tile([128, R_PASS[p], 64], F16, tag=f"xstg{p}",
                          name=f"xstg{p}") for p in range(4)]
        x_r = [pw.tile([128, 8, R_PASS[p] * 64], F16, tag=f"xr{p}",
                       name=f"xr{p}") for p in range(4)]

        # =============== U phase + adjacency ===============
        with tc.tile_pool(name="pu", bufs=1) as pu, \
             tc.tile_pool(name="pue", bufs=3) as pue, \
             tc.tile_pool(name="ppsu", bufs=1, space="PSUM") as ppsu:
            ut_r = pu.tile([128, 8, 18], F32)
            u_r = pu.tile([128, 8, 576], F32)
            liT = pu.tile([128, 4, 64, 9], F32)
            nc.sync.dma_start(
                out=liT[:125].rearrange("p mt n t -> p mt (n t)"),
                in_=dap(li, 0, [(576, 125), (125 * 576, 4), (1, 576)]))
            ws1_sb = pu.tile([128, 4, 2, 128], F32)
            for mt in range(4):
                nc.sync.dma_start(
                    out=ws1_sb[:125, mt],
                    in_=dap(ws1, mt * 125 * 256,
                            [(256, 125), (128, 2), (1, 128)]))
            ws2_sb = pu.tile([128, 2, 128], F32)
            nc.sync.dma_start(
                out=ws2_sb, in_=dap(ws2, 0, [(128, 128), (128 * 128, 2), (1, 128)]))
            bs1_sb = pu.tile([128, 2], F32)
            nc.sync.dma_start(out=bs1_sb, in_=dap(bs1, 0, [(1, 128), (128, 2)]))
            bs2_sb = pu.tile([128, 1], F32)
            nc.sync.dma_start(out=bs2_sb, in_=dap(bs2, 0, [(1, 128), (0, 1)]))
            b_sb = pu.tile([128, 128], F32)
            nc.sync.dma_start(out=b_sb, in_=dap(bmat, 0, [(128, 128), (1, 128)]))
            nmask_sb = pu.tile([128, 64], F32)
            nc.sync.dma_start(out=nmask_sb, in_=nmask[:, :])
            tfT = pu.tile([128, 18, 9], F32)
            nc.sync.dma_start(
                out=tfT[:125],
                in_=dap(tfs, 0, [(9, 125), (125 * 9, 18), (1, 9)]))
            wt1_sb = pu.tile([128, 18, 2, 128], F32)
            for kt in range(18):
                (nc.sync if kt % 2 == 0 else nc.scalar).dma_start(
                    out=wt1_sb[:125, kt],
                    in_=dap(wt1s, kt * 125 * 256,
                            [(256, 125), (128, 2), (1, 128)]))
            bt1_sb = pu.tile([128, 2], F32)
            nc.sync.dma_start(out=bt1_sb, in_=dap(bt1, 0, [(1, 128), (128, 2)]))
            wt2_sb = pu.tile([128, 2, 128], F32)
            nc.sync.dma_start(
                out=wt2_sb, in_=dap(wt2, 0, [(128, 128), (128 * 128, 2), (1, 128)]))
            bt2_sb = pu.tile([128, 1], F32)
            nc.sync.dma_start(out=bt2_sb, in_=dap(bt2, 0, [(1, 128), (0, 1)]))

            # temporal MLP layer 1 partial + mesh AllReduce
            for ct in range(2):
                ps = ppsu.tile([128, 9], F32, tag="ut", bufs=1)
                for kt in range(18):
                    nc.tensor.matmul(ps, wt1_sb[:125, kt, ct, :],
                                     tfT[:125, kt, :],
                                     start=(kt == 0), stop=(kt == 17))
                nc.vector.tensor_copy(ut1p[:, ct, :], ps)
            w_ut = mesh(0, ut1p.rearrange("p a b -> p (a b)"), ut_r)
            ut1r = pu.tile([128, 2, 9], F32)
            sum8(pue, ut_r, ut1r.rearrange("p a b -> p (a b)"), w_ut, 18, "uts")
            ut1a = pu.tile([128, 2, 9], F32)
            for ct in range(2):
                nc.scalar.activation(ut1a[:, ct, :], ut1r[:, ct, :], RELU,
                                     bias=bt1_sb[:, ct:ct + 1])
            utT = pu.tile([128, 9], F32)
            psu = ppsu.tile([128, 9], F32, tag="ut", bufs=1)
            for ct in range(2):
                nc.tensor.matmul(psu, wt2_sb[:, ct, :], ut1a[:, ct, :],
                                 start=(ct == 0), stop=(ct == 1))
            nc.scalar.activation(utT, psu, RELU, bias=bt2_sb)

            # spatial MLP (own nodes)
            us1T = pu.tile([128, 2, 576], F32)
            rhs_li = liT[:125].rearrange("p mt n t -> p mt (n t)")
            for ct in range(2):
                for ch in range(2):
                    ps = ppsu.tile([128, 288], F32, tag="us", bufs=1)
                    for mt in range(4):
                        nc.tensor.matmul(
                            ps, ws1_sb[:125, mt, ct, :],
                            rhs_li[:, mt, 288 * ch:288 * (ch + 1)],
                            start=(mt == 0), stop=(mt == 3))
                    nc.scalar.activation(us1T[:, ct, 288 * ch:288 * (ch + 1)],
                                         ps, RELU, bias=bs1_sb[:, ct:ct + 1])
            usT = pu.tile([128, 576], F32)
            for ch in range(2):
                ps = ppsu.tile([128, 288], F32, tag="us", bufs=1)
                for ct in range(2):
                    nc.tensor.matmul(ps, ws2_sb[:, ct, :],
                                     us1T[:, ct, 288 * ch:288 * (ch + 1)],
                                     start=(ct == 0), stop=(ct == 1))
                nc.scalar.activation(usT[:, 288 * ch:288 * (ch + 1)], ps, RELU,
                                     bias=bs2_sb)
            usT_v = usT.rearrange("p (n t) -> p n t", t=9)
            for ti in range(9):
                nc.vector.tensor_scalar(
                    out=uT_own[:, ti, :], in0=usT_v[:, :, ti],
                    scalar1=utT[:, ti:ti + 1], scalar2=None,
                    op0=mybir.AluOpType.add)
                nc.vector.tensor_tensor(
                    out=uT_own[:, ti, :], in0=uT_own[:, ti, :],
                    in1=nmask_sb, op=mybir.AluOpType.mult)

            # U mesh AllGather (slot-major: [f, slot, (t, n)])
            w_u = mesh(1, uT_own.rearrange("p a b -> p (a b)"), u_r)
            u2_sb = pu.tile([128, 5, 512], F32)
            for ti in range(5):
                cp = nc.vector.tensor_copy(
                    u2_sb[:, ti, :].rearrange("p (c n) -> p c n", c=8),
                    bass.AP(tensor=u_r.tensor,
                            offset=u_r.offset + (2 + ti) * 64,
                            ap=[list(u_r.ap[0]), [576, 8], [1, 64]]))
                gate(cp, w_u)

            # UB^T own
            ubT = pu.tile([128, 9, 64], F32)
            for ti in range(9):
                ps = ppsu.tile([128, 64], F32, tag="ub", bufs=1)
                nc.tensor.matmul(ps, b_sb, uT_own[:, ti, :],
                                 start=True, stop=True)
                nc.vector.tensor_copy(ubT[:, ti, :], ps)

            # adjacency pairs
            for (i1, i2) in PAIRS:
                pid = PAIR_ID[(i1, i2)]
                ti1 = T9IDX[i1]
                t2 = i2 - 28
                sps = ppsu.tile([128, 4, 64], F32, tag="spair", bufs=3)
                for mc, (m0, cnt) in enumerate(MCH):
                    nc.tensor.matmul(sps[:cnt, mc, :],
                                     u2_sb[:, t2, m0:m0 + cnt],
                                     ubT[:, ti1, :], start=True, stop=True)
                msk = pue.tile([128, 4, 64], F32, tag="msk")
                e_sb = pue.tile([128, 4, 64], F32, tag="esb")
                for mc, (m0, cnt) in enumerate(MCH):
                    nc.vector.tensor_scalar(
                        out=msk[:cnt, mc, :], in0=sps[:cnt, mc, :],
                        scalar1=0.05, scalar2=None, op0=mybir.AluOpType.is_ge)
                    nc.vector.tensor_tensor(
                        out=msk[:cnt, mc, :], in0=msk[:cnt, mc, :],
                        in1=sps[:cnt, mc, :], op=mybir.AluOpType.mult)
                    nc.scalar.activation(e_sb[:cnt, mc, :], msk[:cnt, mc, :],
                                         EXP)
                cps = ppsu.tile([1, 64], F32, tag="cs", bufs=1)
                for mc, (m0, cnt) in enumerate(MCH):
                    nc.tensor.matmul(cps, ones_c[:cnt, :], e_sb[:cnt, mc, :],
                                     start=(mc == 0), stop=(mc == 3))
                csh = pue.tile([1, 64], F32, tag="csh")
                nc.vector.tensor_scalar(
                    out=csh, in0=cps, scalar1=-NPAD, scalar2=None,
                    op0=mybir.AluOpType.add)
                rcp = pue.tile([1, 64], F32, tag="rcp")
                nc.vector.reciprocal(rcp, csh)
                rbp = ppsu.tile([128, 64], F32, tag="rb", bufs=1)
                nc.tensor.matmul(rbp, ones_r, rcp, start=True, stop=True)
                for mc, (m0, cnt) in enumerate(MCH):
                    nc.vector.tensor_tensor(
                        out=at_sb[:cnt, pid, mc, :], in0=e_sb[:cnt, mc, :],
                        in1=rbp[:cnt, :], op=mybir.AluOpType.mult)

        # =============== FC3 resident weights ===============
        pw3 = ctx.enter_context(tc.tile_pool(name="pw3", bufs=1))
        w3_sb = pw3.tile([128, 8, W3RES_J, 128], F16)
        for sl in range(4):
            j0 = sl * (W3RES_J // 4)
            (nc.sync if sl % 2 == 0 else nc.scalar).dma_start(
                out=w3_sb[:, :, j0:j0 + W3RES_J // 4, :],
                in_=dap(w3s, j0 * 128,
                        [(8192, 128), (128 * 8192, 8), (128, W3RES_J // 4),
                         (1, 128)]))

        # =============== passes ===============
        ptr = ctx.enter_context(tc.tile_pool(name="ptr", bufs=2))
        pps = ctx.enter_context(tc.tile_pool(name="pps", bufs=1, space="PSUM"))

        w_x = {}
        xT_next = None
        for p in range(4):
            R = R_PASS[p]
            # ---- x^T own tiles [k%128, r, kt_own(32)] ----
            if p == 0:
                xT = ptr.tile([128, 8, 32], F16, tag="xTnA", bufs=1)
                nc.sync.dma_start(
                    out=xT[:, :7, :],
                    in_=dap(obs7t, 0, [(7 * 32, 128), (32, 7), (1, 32)]))
                nc.vector.memset(xT[:, 7, :], 0.0)
            else:
                xT = xT_next            # built during previous pass's conv
            # ---- FC1 (k-sharded, partial sums over own 32 k-tiles) ----
            ps1 = pps.tile([128, 8, 8], F32, tag="fcA", bufs=1,
                           name=f"ps1_{p}")
            for ct in range(8):
                for kt in range(32):
                    nc.tensor.matmul(ps1[:, ct, :R], w1_sb[:, kt, ct, :],
                                     xT[:, :R, kt],
                                     start=(kt == 0), stop=False)
                nc.tensor.matmul(ps1[:, ct, :R], b1r_sb[:, ct, :],
                                 ones16[:, :R], start=False, stop=True)
            nc.vector.tensor_copy(h1p[p], ps1[:, :, :R])
            w_h = mesh(2 + 2 * p, h1p[p].rearrange("p a b -> p (a b)"),
                       h1_r[p])
            h1f = ptr.tile([128, 8, R], F32, tag="h1f", name=f"h1f{p}")
            sum8(ptr, h1_r[p], h1f.rearrange("p a b -> p (a b)"), w_h,
                 8 * R, f"h1s{p}")
            h1T = ptr.tile([128, 8, 8], F16, tag="h1T", name=f"h1T{p}")
            nc.scalar.activation(h1T[:, :, :R], h1f, RELU)
            # ---- FC2 (replicated, local) ----
            ps2 = pps.tile([128, 8, 8], F32, tag="fcA", bufs=1,
                           name=f"ps2_{p}")
            h2T = ptr.tile([128, 8, 8], F16, tag="h2T", name=f"h2T{p}")
            for cto in range(8):
                for kt in range(8):
                    nc.tensor.matmul(ps2[:, cto, :R], w2_sb[:, kt, cto, :],
                                     h1T[:, kt, :R],
                                     start=(kt == 0), stop=False)
                nc.tensor.matmul(ps2[:, cto, :R], b2r_sb[:, cto, :],
                                 ones16[:, :R], start=False, stop=True)
            nc.scalar.activation(h2T[:, :, :R], ps2[:, :, :R], RELU)
            # ---- FC3 ----
            w3t = None
            for jg in range(8):
                b3g = ptr.tile([1, 8, 128], F16, tag="b3g", bufs=2,
                               name=f"b3g_{p}_{jg}")
                nc.scalar.dma_start(
                    out=b3g,
                    in_=dap(b3st, jg * 1024, [(0, 1), (128, 8), (1, 128)]))
                ps3 = pps.tile([128, 8, 8], F32, tag="fc3", bufs=2,
                               name=f"ps3_{p}_{jg}")
                for jj in range(8):
                    j = jg * 8 + jj
                    if j < W3RES_J:
                        wblk = lambda kt, j=j: w3_sb[:, kt, j, :]
                    else:
                        if (j - W3RES_J) % 4 == 0:
                            w3t = ptr.tile([128, 8, 4, 128], F16, tag="w3t",
                                           name=f"w3t{p}_{j}")
                            eng = (nc.sync if ((j - W3RES_J) // 4) % 2 == 0
                                   else nc.scalar)
                            eng.dma_start(
                                out=w3t,
                                in_=dap(w3s, j * 128,
                                        [(8192, 128), (128 * 8192, 8),
                                         (128, 4), (1, 128)]))
                        wblk = (lambda kt, j=j, w3t=w3t:
                                w3t[:, kt, (j - W3RES_J) % 4, :])
                    for kt in range(8):
                        nc.tensor.matmul(ps3[:, jj, :R], wblk(kt),
                                         h2T[:, kt, :R],
                                         start=(kt == 0), stop=False)
                    nc.tensor.matmul(ps3[:, jj, :R], b3g[:, jj, :],
                                     ones16[:, :R], start=False, stop=True)
                nc.scalar.activation(
                    out=xstage[p][:, :, jg * 8:(jg + 1) * 8],
                    in_=rap(ps3, [(1, R), (8, 8)]),
                    func=RELU)
            # ---- X mesh AllGather ----
            w_x[p] = mesh(3 + 2 * p,
                          xstage[p].rearrange("p a b -> p (a b)"), x_r[p])

            # ---- Xk^T tiles [f, slot, n] ----
            need = sorted({(t - k) % L for t in CONV_TS[p] for k in range(3)})
            xkT = {}
            for tv in need:
                sp, slot = XROW[p][tv]
                if sp == 0 and slot == 7 and p != 0:
                    xkT[tv] = zrow_sb
                    continue
                dst = ptr.tile([128, 8, 64], F16, tag=f"xk{tv % 4}",
                               name=f"xk_{p}_{tv}")
                Rs = R_PASS[sp]
                cp = nc.vector.tensor_copy(
                    dst,
                    bass.AP(tensor=x_r[sp].tensor,
                            offset=x_r[sp].offset + slot * 64,
                            ap=[list(x_r[sp].ap[0]), [Rs * 64, 8], [1, 64]]))
                gate(cp, w_x[sp])
                xkT[tv] = dst
            if p == 0:
                cp = nc.vector.tensor_copy(
                    zrow_sb,
                    bass.AP(tensor=x_r[0].tensor,
                            offset=x_r[0].offset + 7 * 64,
                            ap=[list(x_r[0].ap[0]), [8 * 64, 8], [1, 64]]))
                gate(cp, w_x[0])

            # ---- conv units ----
            if p < 3:
                xT_next = ptr.tile([128, 8, 32], F16,
                                   tag=f"xTn{'B' if p % 2 == 0 else 'A'}",
                                   bufs=1, name=f"xTn{p + 1}")
            for t in CONV_TS[p]:
                relu_parts = []
                for k in range(3):
                    tv = (t - k) % L
                    xk = xkT[tv].rearrange("p c n -> p (c n)")
                    if k == 0:
                        dirs = [(PAIR_ID[(t, t)], 0, 64)]
                        wsl = wfb_sb[:, 0, :]
                        ow = 64
                    else:
                        dirs = [(PAIR_ID[((t - k) % L, t)], 0, 64),
                                (PAIR_ID[((t + k) % L, t)], 64, 128)]
                        wsl = wfb_sb.rearrange("p d o -> p (d o)")[
                            :, (2 * k - 1) * 64:(2 * k + 1) * 64]
                        ow = 128
                    y_ps = pps.tile([128, 4, 128], F32, tag="yps", bufs=2,
                                    name=f"yps{p}_{t}_{k}")
                    y_sb = ptr.tile([128, 4, 128], F16, tag="ysb",
                                    name=f"ysb{p}_{t}_{k}")
                    for mc, (m0, cnt) in enumerate(MCH):
                        nc.tensor.matmul(y_ps[:cnt, mc, :ow],
                                         xk[:, m0:m0 + cnt],
                                         wsl, start=True, stop=True)
                    nc.vector.tensor_copy(y_sb[:, :, :ow], y_ps[:, :, :ow])
                    hps = pps.tile([64, 64], F32, tag="hps", bufs=2,
                                   name=f"hps{p}_{t}_{k}")
                    for mc, (m0, cnt) in enumerate(MCH):
                        for (pid, o0, o1) in dirs:
                            nc.tensor.matmul(
                                hps, at_sb[:cnt, pid, mc, :],
                                y_sb[:cnt, mc, o0:o1],
                                start=(mc == 0 and o0 == 0), stop=False)
                    nc.tensor.matmul(hps, ones64[:, :64], bcr_sb,
                                     start=False, stop=True)
                    rk = ptr.tile([64, 64], F32, tag=f"rk{k}",
                                  name=f"rk{p}_{t}_{k}")
                    nc.scalar.activation(rk, hps, RELU)
                    relu_parts.append(rk)
                zacc = ptr.tile([64, 64], F32, tag="zacc",
                                name=f"zacc{p}_{t}")
                nc.vector.tensor_tensor(out=zacc, in0=relu_parts[0],
                                        in1=relu_parts[1],
                                        op=mybir.AluOpType.add)
                nc.vector.tensor_tensor(out=zacc, in0=zacc,
                                        in1=relu_parts[2],
                                        op=mybir.AluOpType.add)
                if t == 32:
                    nc.sync.dma_start(
                        out=dap(out_ext, p * NODES_PER_CORE * DO,
                                [(64, 64), (1, 64)]),
                        in_=zacc)
                if p < 3:
                    # feed next pass's x^T (own nodes only -- no Z exchange)
                    r = CONV_TS[p].index(t)
                    zc16 = ptr.tile([64, 64], F16, tag="zc16", bufs=1,
                                    name=f"zc16_{p}_{t}")
                    nc.vector.tensor_copy(zc16, zacc)
                    tpz = pps.tile([64, 64], F16, tag="tp", bufs=1,
                                   name=f"tpz{p}_{t}")
                    nc.tensor.transpose(tpz, zc16, ident[:64, :64])
                    # tpz = zacc^T [o, n]; node-pair j -> partitions (o, o+64)
                    nc.vector.tensor_copy(xT_next[0:64, r, :],
                                          rap(tpz, [(2, 32)]))
                    nc.vector.tensor_copy(xT_next[64:128, r, :],
                                          bass.AP(tensor=tpz.tensor,
                                                  offset=tpz.offset + 1,
                                                  ap=[list(tpz.ap[0]),
                                                      [2, 32]]))

    nc.finalize()
    return nc


# ======================= host side =======================
_NC_CACHE = {}


def _get_nc():
    if "nc" not in _NC_CACHE:
        _NC_CACHE["nc"] = build()
    return _NC_CACHE["nc"]


def make_in_maps(inputs):
    obs = np.asarray(inputs["observation"], np.float32)
    tf = np.asarray(inputs["time_feats"], np.float32)
    lin = np.asarray(inputs["layer_initial"], np.float32)
    Wf = np.asarray(inputs["Wf"], np.float32)
    Wb = np.asarray(inputs["Wb"], np.float32)
    wfb = np.ascontiguousarray(
        np.stack([Wf[0] + Wb[0], Wf[1], Wb[1], Wf[2], Wb[2]])).astype(np.float16)
    w3 = np.asarray(inputs["Wfc3"], np.float32)
    b3 = np.asarray(inputs["bfc3"], np.float32)
    o7 = obs[26:33]                                   # (7, 32000)
    o7p = np.zeros((7, 256, 128), np.float32)
    o7p[:, :KT1, :] = o7.reshape(7, KT1, 128)
    obs7t_full = o7p.transpose(2, 0, 1).astype(np.float16)  # (128, 7, 256)
    w1full = np.asarray(inputs["Wfc1"], np.float32)   # (32000, 1024)
    b1 = np.asarray(inputs["bfc1"], np.float32)
    b2 = np.asarray(inputs["bfc2"], np.float32)
    w2full = np.ascontiguousarray(np.asarray(inputs["Wfc2"], np.float32))
    b1st = np.ascontiguousarray((b1 / NC).reshape(1, -1)).astype(np.float16)
    b2st = np.ascontiguousarray(b2.reshape(1, -1)).astype(np.float16)
    kfn = KTF // NC
    in_maps = []
    for c in range(NC):
        n0, cnt = NODE0[c], REAL_NODES[c]
        li = np.zeros((N, NODES_PER_CORE, 9), np.float32)
        li[:, :cnt, :] = lin[n0:n0 + cnt][:, T9, :].transpose(2, 0, 1)
        nmask = np.zeros((128, NODES_PER_CORE), np.float32)
        nmask[:, :cnt] = 1.0
        w3s = np.zeros((FC2W, 8192), np.float16)
        b3s = np.zeros((8192,), np.float16)
        c0, c1 = 8192 * c, min(8192 * (c + 1), 64000)
        w3s[:, :c1 - c0] = w3[:, c0:c1]
        b3s[:c1 - c0] = b3[c0:c1]
        b3st = np.ascontiguousarray(b3s.reshape(1, -1))
        w1row = np.zeros((4096, 1024), np.float16)
        k0, k1 = 4096 * c, min(4096 * (c + 1), 32000)
        w1row[:k1 - k0] = w1full[k0:k1]
        kf0 = kfn * c
        in_maps.append({
            "li": li,
            "tfs": np.ascontiguousarray(tf[T9][:, kf0:kf0 + kfn].T),
            "obs7t": np.ascontiguousarray(obs7t_full[:, :, 32 * c:32 * (c + 1)]),
            "nmask": nmask,
            "ws1": np.asarray(inputs["Ws1"], np.float32),
            "bs1": np.asarray(inputs["bs1"], np.float32),
            "ws2": np.asarray(inputs["Ws2"], np.float32),
            "bs2": np.asarray(inputs["bs2"], np.float32),
            "wt1s": np.ascontiguousarray(
                np.asarray(inputs["Wt1"], np.float32)[kf0:kf0 + kfn]),
            "bt1": np.asarray(inputs["bt1"], np.float32),
            "wt2": np.asarray(inputs["Wt2"], np.float32),
            "bt2": np.asarray(inputs["bt2"], np.float32),
            "bmat": np.asarray(inputs["B"], np.float32),
            "w1s": w1row,
            "b1st": b1st,
            "w2s": w2full.astype(np.float16),
            "b2st": b2st,
            "w3s": w3s,
            "b3st": b3st,
            "wfb": wfb,
            "bconv": np.asarray(inputs["bconv"], np.float16),
        })
    return in_maps


def _assemble(results):
    out = np.zeros((4, N, DO), np.float32)
    for c in range(NC):
        n0, cnt = NODE0[c], REAL_NODES[c]
        out[:, n0:n0 + cnt, :] = results[c]["out"][:, :cnt, :]
    return out


def kernel(**inputs):
    nc = _get_nc()
    in_maps = make_in_maps(inputs)
    res = run_bass_kernel_spmd(nc, in_maps, core_ids=list(range(NC)))
    return _assemble(res.results)
